# revision 36
# baseline (speedup 1.0000x reference)
"""GCN encoder (2x GCNConv + MLP proj head) on 8 Trainium2 NeuronCores.

Strategy: shard nodes across the 8 cores (1250/core, padded to 1280).
The symmetric GCN norm dis[src]*dis[dst] factors into per-node pre/post
scaling, so each aggregation round is: per-core dense matmul (X@W, bf16,
f32 PSUM) + dis-scale -> AllGather of the scaled features -> per 128-dst
window: dma_gather of deduped source rows + host-precomputed one-hot
scatter matmuls accumulating segment sums in PSUM -> self-loop term on
the DVE -> dis post-scale on the scalar engine.

Both feature AllGathers run in HALVES bound to SEPARATE dram tensors
(h?A/h?B): the first h1 half fires right after phase-A window 4 (it
absorbs the runtime's first-CC barrier, which ends only ~15-25us after
the first trigger), and each window's gather pieces are split at the
A/B source boundary (host pads each region per core) so A-pieces gate
only on the A AllGather. Per-window chunk counts are variable
(max-over-cores per half) instead of one global k.

Round 2 runs in fp8e4: h2' is quantized once (out rel err ~1.8e-2 <
the 2e-2 gate), halving the h2 AllGathers and round-2 gather bytes,
with DoubleRow scatter matmuls at 2x PE rate. The S table ships as fp8
(counts exact) and is DVE-cast to bf16 per window for round 1. The
proj head's matmuls interleave into round-2 windows where the PE would
otherwise idle on gathers.
"""
import json

import numpy as np
import ml_dtypes

N = 10000
E = 160000
D = 512
NC = 8
NPC = N // NC  # 1250 nodes per core
CH = 10  # 128-node chunks / windows per core
NPAD = CH * 128  # 1280
HALF = NPAD // 2  # 640

_BF16 = ml_dtypes.bfloat16
_F8 = ml_dtypes.float8_e4m3

_WAIT_SPLIT_DONE = False


def _install_wait_split():
    """This container's walrus rejects instructions with >1 sync wait.
    Hoist extra waits onto single-wait Drain instructions just before the
    instruction on the same engine (same sequencer => same semantics)."""
    global _WAIT_SPLIT_DONE
    if _WAIT_SPLIT_DONE:
        return
    _WAIT_SPLIT_DONE = True
    import concourse.bass as bass

    orig = bass.Bass.to_json_bytes

    def _split_block(instructions):
        out = []
        changed = False
        for inst in instructions:
            sync = inst.get("sync_info")
            waits = (sync or {}).get("on_wait") or []
            if len(waits) > 1:
                changed = True
                for j, w in enumerate(waits[:-1]):
                    out.append(
                        {
                            "engine": inst["engine"],
                            "ins": [],
                            "name": f"{inst['name']}-wsplit{j}",
                            "opcode": "Drain",
                            "outs": [],
                            "sync_info": {"on_update": [], "on_wait": [w]},
                        }
                    )
                sync["on_wait"] = waits[-1:]
            out.append(inst)
        return out, changed

    def to_json_bytes(self):
        js = json.loads(orig(self))
        stack = [js]
        while stack:
            d = stack.pop()
            if isinstance(d, dict):
                if "instructions" in d:
                    new, changed = _split_block(d["instructions"])
                    if changed:
                        d["instructions"] = new
                for v in d.values():
                    if isinstance(v, (dict, list)):
                        stack.append(v)
            elif isinstance(d, list):
                stack.extend(d)
        return json.dumps(js).encode()

    bass.Bass.to_json_bytes = to_json_bytes


def _pieces_for(nach, nbch, gmax):
    """Gather pieces for one window: (chunk_offset, nchunks, half) with A
    chunks [0, nach) then B chunks [nach, nach+nbch), grouped by gmax."""
    out = []
    a = 0
    while a < nach:
        nk = min(gmax, nach - a)
        out.append((a, nk, 0))
        a += nk
    tot = nach + nbch
    while a < tot:
        nk = min(gmax, tot - a)
        out.append((a, nk, 1))
        a += nk
    return out


def _build_program(kws, nachs, nbchs, has_b1, has_b2, has_bp1, has_bp2):
    import concourse.bass as bass
    import concourse.tile as tile
    from concourse import mybir
    from concourse.library_config import mlp
    from concourse.library_overlay import lower_extended_insts
    from concourse.tile_rust import add_dep_helper

    f32 = mybir.dt.float32
    bf16 = mybir.dt.bfloat16
    f8 = mybir.dt.float8e4
    i16 = mybir.dt.int16
    ACTF = mybir.ActivationFunctionType

    TOT = sum(kws)
    KMAX = max(kws)
    offs = np.concatenate([[0], np.cumsum(kws)])[:-1].tolist()
    # round-1: 2-chunk (256-desc, 256KB) pieces; round-2 fp8: 4-chunk
    # (512-desc, 256KB). Bigger pieces hit SWDGE data backpressure.
    pieces1 = [_pieces_for(nachs[w], nbchs[w], 2) for w in range(CH)]
    pieces2 = [_pieces_for(nachs[w], nbchs[w], 4) for w in range(CH)]

    nc = bass.Bass(num_swdge_queues=4)

    # ---- external inputs (per-core layouts prepared on host) ----
    xt_ext = nc.dram_tensor("xt", [128, 4 * NPAD], bf16, kind="ExternalInput")
    w_ext = {
        nm: nc.dram_tensor(nm, [128, 4 * D], bf16, kind="ExternalInput")
        for nm in ("w1t", "w2t", "wp1t", "wp2t")
    }
    dis_ext = nc.dram_tensor("dis", [128, CH], f32, kind="ExternalInput")
    idx_ext = nc.dram_tensor("idx16", [128, TOT * 8], i16, kind="ExternalInput")
    # S (one-hot counts) ships as fp8e4 (counts are small ints, exact):
    # round 2 consumes it directly (fp8 DoubleRow matmuls); round 1 casts
    # per-window to bf16 on the DVE. Halves the dominant input load.
    s_ext = nc.dram_tensor("stab", [128, TOT * 128], f8, kind="ExternalInput")
    ident_ext = nc.dram_tensor("ident", [128, 128], bf16, kind="ExternalInput")
    b_ext = {}
    for nm, has in (
        ("b1", has_b1),
        ("b2", has_b2),
        ("bp1", has_bp1),
        ("bp2", has_bp2),
    ):
        if has:
            b_ext[nm] = nc.dram_tensor(nm, [128, D], f32, kind="ExternalInput")

    # ---- external outputs ----
    # z in bf16: quantization ~2^-9 rel, far under the 2e-2 gate.
    z_out = nc.dram_tensor("z", [NPAD, D], bf16, kind="ExternalOutput")
    out_out = nc.dram_tensor("agg", [NPAD, D], f32, kind="ExternalOutput")
    proj_out = nc.dram_tensor("proj", [NPAD, D], bf16, kind="ExternalOutput")

    # ---- internal DRAM ----
    # Separate tensors per AllGather half: exact Tile deps (A-pieces gate
    # only on the A AllGather; the AG trigger waits only its own shard).
    h1sA = nc.dram_tensor("h1sA", [HALF, D], bf16)
    h1sB = nc.dram_tensor("h1sB", [HALF, D], bf16)
    h1A = nc.dram_tensor("h1A", [NC * HALF, D], bf16, addr_space="Shared")
    h1B = nc.dram_tensor("h1B", [NC * HALF, D], bf16, addr_space="Shared")
    h2sA = nc.dram_tensor("h2sA", [HALF, D], f8)
    h2sB = nc.dram_tensor("h2sB", [HALF, D], f8)
    h2A = nc.dram_tensor("h2A", [NC * HALF, D], f8, addr_space="Shared")
    h2B = nc.dram_tensor("h2B", [NC * HALF, D], f8, addr_space="Shared")
    dum_in = nc.dram_tensor("dum_in", [128, 2], f32)
    dum_out = nc.dram_tensor("dum_out", [128, 16], f32, addr_space="Shared")

    core_ids = list(range(NC))

    with tile.TileContext(nc) as tc:
        with (
            tc.tile_pool(name="const", bufs=1) as cpool,
            tc.tile_pool(name="work", bufs=3) as wpool,
            tc.tile_pool(name="gat", bufs=6) as gpool,
            tc.tile_pool(name="sc", bufs=3) as scpool,
            tc.tile_pool(name="tp", bufs=1) as tpool,
            tc.tile_pool(name="psA", bufs=2, space="PSUM") as psA,
            tc.tile_pool(name="psB", bufs=4, space="PSUM") as psB,
        ):
            # Dep-free dummy first collective, triggered ~11us in: the
            # runtime's first-CC barrier ends only ~25us after the first
            # trigger, so binding it to this tiny AllGather instead of the
            # h1 AG-A (trigger ~35us) starts the real AGs ~25us earlier.
            dum_t = cpool.tile([128, 2], f32, name="dum_sb")
            nc.vector.memset(dum_t[:], 0.0)
            nc.sync.dma_start(dum_in[:], dum_t[:])
            nc.gpsimd.collective_compute(
                "AllGather",
                mybir.AluOpType.bypass,
                ins=[dum_in[:]],
                outs=[dum_out[:]],
                replica_groups=[list(range(NC))],
            )
            lib_inst = nc.gpsimd.load_library(mlp)
            # one shared register per distinct gather size (to_reg per call
            # would exhaust the Pool register file at 60 gathers)
            sizes = sorted(
                {nk for pl in pieces1 + pieces2 for (_, nk, _) in pl}
            )
            nidx_regs = {nk: nc.gpsimd.to_reg(nk * 128) for nk in sizes}

            # ---- phase-A-critical loads, split across the sync + scalar
            # HWDGE rings (each sustains only ~65-115GB/s): w1t + xt halves
            # + dis land first so the first matmul can go ~15us in.
            w_t = {}
            w_t["w1t"] = cpool.tile([128, 4 * D], bf16, tag="w1t", name="w1t")
            nc.sync.dma_start(w_t["w1t"][:], w_ext["w1t"][:])
            # xt lives in a gather-pool buffer: it is dead after phase A,
            # and the pool's WAR rotation hands the buffer to a round-1
            # window gather afterwards (frees 1.25MB of persistent SBUF).
            xt_buf = gpool.tile([128, 16, D], bf16, tag="g", name="xtbuf")
            xt_t = xt_buf[:].rearrange("p k d -> p (k d)")
            XH = 2 * NPAD
            nc.scalar.dma_start(xt_t[:, :XH], xt_ext[:, :XH])
            nc.sync.dma_start(xt_t[:, XH : 4 * NPAD], xt_ext[:, XH:])
            dis_t = cpool.tile([128, CH], f32)
            nc.sync.dma_start(dis_t[:], dis_ext[:])
            # ---- bulk loads on the scalar HWDGE ring (parallel to above;
            # keep the gpsimd queue free so the AG trigger isn't delayed) ----
            idx_t = cpool.tile([128, TOT * 8], i16)
            nc.scalar.dma_start(idx_t[:], idx_ext[:])
            s8_t = cpool.tile([128, TOT * 128], f8)
            nc.scalar.dma_start(s8_t[:], s_ext[:])
            ident_t = cpool.tile([128, 128], bf16)
            nc.scalar.dma_start(ident_t[:], ident_ext[:])
            for nm in ("w2t", "wp1t", "wp2t"):
                w_t[nm] = cpool.tile([128, 4 * D], bf16, tag=nm, name=nm)
                nc.scalar.dma_start(w_t[nm][:], w_ext[nm][:])
            b_t = {}
            for nm in b_ext:
                b_t[nm] = cpool.tile([128, D], f32, tag=nm, name=nm + "_bc")
                nc.scalar.dma_start(b_t[nm][:], b_ext[nm][:])

            # persistent scaled-feature chunks (self-loop term source)
            h1p_t = cpool.tile([128, CH * D], bf16)
            h2p_t = cpool.tile([128, CH * D], bf16)

            def dense_layer(lhs_tiles, w_name, m, dep=None):
                ps = psA.tile([128, D], f32, tag="dense")
                for kk in range(4):
                    mi = nc.tensor.matmul(
                        ps[:],
                        lhs_tiles(kk, m),
                        w_t[w_name][:, kk * D : (kk + 1) * D],
                        start=(kk == 0),
                        stop=(kk == 3),
                    )
                    if kk == 0 and dep is not None:
                        add_dep_helper(
                            getattr(mi, "ins", mi),
                            getattr(dep, "ins", dep),
                            reason="pin proj head to its window",
                        )
                return ps

            def xt_tile(kk, m):
                return xt_t[:, kk * NPAD + m * 128 : kk * NPAD + (m + 1) * 128]

            def scale_to(dst_ap, ps, m, bias_name):
                """dst = dis_m * (ps + bias) via ACT (bias pre-add on DVE).
                Returns the pre-scale source for further ACT copies."""
                if bias_name in b_t:
                    tmp = wpool.tile([128, D], f32, tag="btmp")
                    nc.vector.tensor_tensor(
                        tmp[:], ps[:], b_t[bias_name][:], op=mybir.AluOpType.add
                    )
                    src = tmp
                else:
                    src = ps
                nc.scalar.activation(
                    dst_ap, src[:], ACTF.Copy, scale=dis_t[:, m : m + 1]
                )
                return src

            # ---- phase A: H1' = dis * (X @ W1 + b1), own nodes ----
            # Windows 0-4 land in h1sA, 5-9 in h1sB; the A AllGather fires
            # right after window 4 (absorbs the first-CC barrier).
            for m in range(CH):
                ps = dense_layer(xt_tile, "w1t", m)
                scale_to(h1p_t[:, m * D : (m + 1) * D], ps, m, "b1")
                sh, r0 = (h1sA, m * 128) if m < 5 else (h1sB, (m - 5) * 128)
                nc.sync.dma_start(
                    sh[r0 : r0 + 128, :], h1p_t[:, m * D : (m + 1) * D]
                )
                if m == 4:
                    nc.gpsimd.collective_compute(
                        "AllGather",
                        mybir.AluOpType.bypass,
                        ins=[h1sA[0:HALF, :]],
                        outs=[h1A[0 : NC * HALF, :]],
                        replica_groups=[core_ids],
                    )
            nc.gpsimd.collective_compute(
                "AllGather",
                mybir.AluOpType.bypass,
                ins=[h1sB[0:HALF, :]],
                outs=[h1B[0 : NC * HALF, :]],
                replica_groups=[core_ids],
            )

            def one_gather(src_ap, off, a, nk, g_ap, qn):
                """Gather chunks [a, a+nk) of a window's table (flat chunk
                offset `off`+a in the idx/S tables) into g_ap."""
                gi = nc.gpsimd.dma_gather(
                    g_ap,
                    src_ap,
                    idx_t[:, (off + a) * 8 : (off + a + nk) * 8],
                    num_idxs=nk * 128,
                    num_idxs_reg=nidx_regs[nk],
                    elem_size=D,
                    single_packet=True,
                    queue_num=qn,
                )
                add_dep_helper(
                    getattr(gi, "ins", gi),
                    getattr(lib_inst, "ins", lib_inst),
                    reason="mlp library before dma_gather",
                )

            def issue_gathers(srcA, srcB, w, g_t, pieces):
                for hh, (a, nk, hb) in enumerate(pieces[w]):
                    src = srcA[:] if hb == 0 else srcB[:]
                    one_gather(
                        src, offs[w], a, nk, g_t[:, a : a + nk, :], hh % 4
                    )

            def self_add(ps, selfsrc_t, w):
                """Self-loop term on DVE (frees the PE of identity matmuls)."""
                tmp = wpool.tile([128, D], f32, tag="selfadd", bufs=3)
                nc.vector.tensor_tensor(
                    tmp[:],
                    ps[:],
                    selfsrc_t[:, w * D : (w + 1) * D],
                    op=mybir.AluOpType.add,
                )
                return tmp

            def agg_round1(srcA, srcB, selfsrc_t):
                """bf16 scatter: per window, DVE-cast the fp8 S chunk to bf16
                then accumulate k_w chunk matmuls."""
                for w in range(CH):
                    k_w = kws[w]
                    g_t = gpool.tile([128, KMAX, D], bf16, tag="g", name=f"g{w}")
                    issue_gathers(srcA, srcB, w, g_t, pieces1)
                    s_c = scpool.tile(
                        [128, KMAX * 128], bf16, tag="sc", name=f"sc{w}"
                    )
                    nc.vector.tensor_copy(
                        s_c[:, : k_w * 128],
                        s8_t[:, offs[w] * 128 : (offs[w] + k_w) * 128],
                    )
                    ps = psB.tile([128, D], f32, tag="agg", name=f"agg{w}")
                    for k in range(k_w):
                        nc.tensor.matmul(
                            ps[:],
                            s_c[:, k * 128 : (k + 1) * 128],
                            g_t[:, k, :],
                            start=(k == 0),
                            stop=(k == k_w - 1),
                        )
                    yield w, self_add(ps, selfsrc_t, w)

            s8_pairs = s8_t[:].rearrange("p (c k) -> p c k", k=128)

            def agg_round2(srcA, srcB, selfsrc_t, pre=None):
                """fp8 DoubleRow scatter: pairs of 128-src chunks per matmul
                at 2x PE rate; gathers move half the bytes."""
                g_full = None
                for w in range(CH):
                    k_w = kws[w]
                    # fp8 tiles are half the round-1 bf16 size: pack TWO
                    # windows per shared-tag buffer, so all 10 round-2
                    # windows fit in 5 bufs with no rotation — every gather
                    # can pre-issue as soon as the h2 AG halves land.
                    if w % 2 == 0:
                        g_full = gpool.tile(
                            [128, 2 * KMAX, D], f8, tag="g", name=f"g8_{w}"
                        )
                    hw_ = (w % 2) * KMAX
                    g_t = g_full[:, hw_ : hw_ + KMAX, :]
                    issue_gathers(srcA, srcB, w, g_t, pieces2)
                    if pre is not None:
                        pre(w)
                    ps = psB.tile([128, D], f32, tag="agg", name=f"agg{w}")
                    np_ = k_w // 2
                    for c in range(np_):
                        nc.tensor.matmul(
                            ps[:],
                            s8_pairs[
                                :, offs[w] + 2 * c : offs[w] + 2 * c + 2, :
                            ],
                            g_t[:, 2 * c : 2 * c + 2, :],
                            start=(c == 0),
                            stop=(c == np_ - 1),
                            perf_mode=mybir.MatmulPerfMode.DoubleRow,
                        )
                    yield w, self_add(ps, selfsrc_t, w)

            # ---- fused round 1 + L2, pipelined per window ----
            # zt window-major [128, w, kk, 128]
            zt_t = tpool.tile([128, CH * 4 * 128], bf16, tag="zt")
            zt_v = zt_t[:].rearrange("p (w k n) -> p w k n", k=4, n=128)

            def proj_head(w):
                ps3 = dense_layer(lambda kk, m: zt_v[:, m, kk, :], "wp1t", w)
                p1_b = wpool.tile([128, D], bf16, tag="p1")
                if "bp1" in b_t:
                    btmp = wpool.tile([128, D], f32, tag="btmp")
                    nc.vector.tensor_tensor(
                        btmp[:], ps3[:], b_t["bp1"][:], op=mybir.AluOpType.add
                    )
                    nc.vector.tensor_scalar(
                        p1_b[:], btmp[:], 0.0, None, op0=mybir.AluOpType.max
                    )
                else:
                    nc.vector.tensor_scalar(
                        p1_b[:], ps3[:], 0.0, None, op0=mybir.AluOpType.max
                    )
                psT2 = psA.tile([128, 4, 128], bf16, tag="tr", name=f"tr2{w}")
                for kk in range(4):
                    nc.tensor.transpose(
                        psT2[:, kk, :],
                        p1_b[:, kk * 128 : (kk + 1) * 128],
                        ident_t[:],
                    )
                # p1^T is consumed immediately by proj2 — rotating tile
                p1c = scpool.tile([128, 4, 128], bf16, tag="p1c", name=f"p1c{w}")
                nc.vector.tensor_copy(p1c[:], psT2[:])
                ps4 = dense_layer(lambda kk, m: p1c[:, kk, :], "wp2t", w)
                pj_t = wpool.tile([128, D], bf16, tag="pj")
                if "bp2" in b_t:
                    nc.vector.tensor_tensor(
                        pj_t[:], ps4[:], b_t["bp2"][:], op=mybir.AluOpType.add
                    )
                else:
                    nc.vector.tensor_copy(pj_t[:], ps4[:])
                nc.sync.dma_start(proj_out[w * 128 : (w + 1) * 128, :], pj_t[:])

            for w, tmp in agg_round1(h1A, h1B, h1p_t):
                z_b = wpool.tile([128, D], bf16, tag="zb")
                nc.scalar.activation(
                    z_b[:], tmp[:], ACTF.Copy, scale=dis_t[:, w : w + 1]
                )
                nc.sync.dma_start(z_out[w * 128 : (w + 1) * 128, :], z_b[:])
                # transpose z chunk into zt columns; relu'd copy into rt
                psT = psA.tile([128, 4, 128], bf16, tag="tr", name=f"tr{w}")
                for kk in range(4):
                    nc.tensor.transpose(
                        psT[:, kk, :],
                        z_b[:, kk * 128 : (kk + 1) * 128],
                        ident_t[:],
                    )
                zt_cols = zt_v[:, w, :, :]
                # rt (relu'd z^T) is consumed by L2 in this same window, so a
                # rotating tile suffices (saves 2.5MB of SBUF vs NPAD-wide)
                rt_w = wpool.tile([128, 4, 128], bf16, tag="rtw")
                nc.vector.tensor_copy(zt_cols, psT[:])
                nc.vector.tensor_scalar(
                    rt_w[:], psT[:], 0.0, None, op0=mybir.AluOpType.max
                )
                # L2 for this node chunk -> H2' shard (bf16 self copy + fp8
                # collective copy)
                ps2 = dense_layer(lambda kk, m: rt_w[:, kk, :], "w2t", w)
                src2 = scale_to(h2p_t[:, w * D : (w + 1) * D], ps2, w, "b2")
                h2f8 = wpool.tile([128, D], f8, tag="h2f8")
                nc.scalar.activation(
                    h2f8[:], src2[:], ACTF.Copy, scale=dis_t[:, w : w + 1]
                )
                sh2, r0 = (h2sA, w * 128) if w < 5 else (h2sB, (w - 5) * 128)
                nc.sync.dma_start(sh2[r0 : r0 + 128, :], h2f8[:])
                if w == 4:
                    nc.gpsimd.collective_compute(
                        "AllGather",
                        mybir.AluOpType.bypass,
                        ins=[h2sA[0:HALF, :]],
                        outs=[h2A[0 : NC * HALF, :]],
                        replica_groups=[core_ids],
                    )
            nc.gpsimd.collective_compute(
                "AllGather",
                mybir.AluOpType.bypass,
                ins=[h2sB[0:HALF, :]],
                outs=[h2B[0 : NC * HALF, :]],
                replica_groups=[core_ids],
            )

            # ---- phase E: round 2 -> out, proj head interleaved per window
            # (proj depends only on zt, so its matmuls fill the tensor engine
            # while the h2 AllGather half and the window's gathers land) ----
            for w, tmp in agg_round2(h2A, h2B, h2p_t, pre=proj_head):
                o_f = wpool.tile([128, D], f32, tag="of")
                nc.scalar.activation(
                    o_f[:], tmp[:], ACTF.Copy, scale=dis_t[:, w : w + 1]
                )
                nc.sync.dma_start(out_out[w * 128 : (w + 1) * 128, :], o_f[:])

    lower_extended_insts(nc)
    return nc


def _host_prep(x, edge_index, W1, W2, Wp1, Wp2):
    src = np.asarray(edge_index[0], np.int64)
    dst = np.asarray(edge_index[1], np.int64)

    # degree includes self loops (norm definition), but self edges are
    # handled on-device via the self-add, not the gather.
    deg = (np.bincount(np.concatenate([dst, np.arange(N)]), minlength=N)).astype(
        np.float32
    )
    dis = (1.0 / np.sqrt(np.maximum(deg, 1.0))).astype(np.float32)

    owner = src // NPC
    local = src - owner * NPC
    # AllGather halves land rank-major per half: row = owner*HALF + local
    # within each half tensor (A: local<HALF; B: local-HALF).
    in_b = local >= HALF
    gather_row = np.where(
        in_b, NC * HALF + owner * HALF + (local - HALF), owner * HALF + local
    )

    dst_core = dst // NPC
    dst_local = dst - dst_core * NPC  # [0, 1250)
    win = dst_local // 128
    dloc = dst_local - win * 128

    order = np.lexsort((dst_local, dst_core))
    g_sorted = gather_row[order]
    dc = dst_core[order]
    wn = win[order]
    dl = dloc[order]

    counts = np.zeros((NC, CH), np.int64)
    np.add.at(counts, (dc, wn), 1)
    flat_counts = counts.reshape(-1)
    starts = np.concatenate([[0], np.cumsum(flat_counts)])[:-1].reshape(NC, CH)

    # dedup per (core, window); split at the A/B half boundary (sources
    # sorted ascending so A rows come first)
    uniq = {}
    cntA = np.zeros((NC, CH), np.int64)
    cntB = np.zeros((NC, CH), np.int64)
    for c in range(NC):
        for w in range(CH):
            s0, n = starts[c, w], counts[c, w]
            rows = g_sorted[s0 : s0 + n]
            dd = dl[s0 : s0 + n]
            u, inv = np.unique(rows, return_inverse=True)
            a = int(np.searchsorted(u, NC * HALF))
            uniq[(c, w)] = (u, inv, dd, a)
            cntA[c, w] = a
            cntB[c, w] = len(u) - a

    # per-window shared chunk counts: each half padded (duplicate row 0,
    # zero S columns) to the max over cores; total kept even for the
    # round-2 DoubleRow chunk pairs.
    nachs, nbchs, kws = [], [], []
    for w in range(CH):
        na = int(np.ceil(cntA[:, w].max() / 128))
        nb = int(np.ceil(cntB[:, w].max() / 128))
        if (na + nb) % 2:
            nb += 1
        nachs.append(na)
        nbchs.append(nb)
        kws.append(na + nb)
    offs = np.concatenate([[0], np.cumsum(kws)])[:-1]
    TOT = int(sum(kws))

    per_core = []
    for c in range(NC):
        idx_flat = np.zeros((TOT * 128,), np.int64)
        s_tab = np.zeros((TOT * 128, 128), np.float32)
        for w in range(CH):
            u, inv, dd, a = uniq[(c, w)]
            base = offs[w] * 128
            nA = nachs[w] * 128
            # A sources at [base, base+lenA), B at [base+nA, ...): indices
            # rebased per half tensor; inv remapped to the padded layout.
            idx_flat[base : base + a] = u[:a]
            idx_flat[base + nA : base + nA + (len(u) - a)] = (
                u[a:] - NC * HALF
            )
            pos = np.where(inv < a, inv, nA + (inv - a))
            np.add.at(s_tab, (base + pos, dd), 1.0)

        wlen16 = TOT * 128 // 16
        iw = idx_flat.reshape(wlen16, 16).T  # [16, TOT*8]
        idx16 = np.ascontiguousarray(
            np.tile(iw, (8, 1)).astype(np.int16)
        )

        # stab: [128, TOT*128]; col (off+k)*128+d, part p = S[(off+k)*128+p, d]
        stab = (
            s_tab.reshape(TOT, 128, 128).transpose(1, 0, 2).reshape(128, -1)
        )
        stab = np.ascontiguousarray(stab).astype(_F8)

        xc = np.zeros((NPAD, D), np.float32)
        xc[:NPC] = x[c * NPC : (c + 1) * NPC]
        xt = xc.T.reshape(4, 128, NPAD).transpose(1, 0, 2).reshape(128, -1)
        xt = np.ascontiguousarray(xt).astype(_BF16)

        dis_c = np.zeros((NPAD,), np.float32)
        dis_c[:NPC] = dis[c * NPC : (c + 1) * NPC]
        dis_t = np.ascontiguousarray(dis_c.reshape(CH, 128).T, np.float32)

        per_core.append(
            {"xt": xt, "idx16": idx16, "stab": stab, "dis": dis_t}
        )

    def wtile(W):
        wt = (
            np.asarray(W, np.float32)
            .reshape(4, 128, D)
            .transpose(1, 0, 2)
            .reshape(128, -1)
        )
        return np.ascontiguousarray(wt).astype(_BF16)

    shared = {
        "w1t": wtile(W1),
        "w2t": wtile(W2),
        "wp1t": wtile(Wp1),
        "wp2t": wtile(Wp2),
        "ident": np.eye(128, dtype=np.float32).astype(_BF16),
    }
    return kws, nachs, nbchs, per_core, shared


def run(inputs, trace=False, **run_kwargs):
    """Build + run; returns ((out, z, proj), BassKernelResults)."""
    _install_wait_split()
    from concourse.bass_utils import run_bass_kernel_spmd

    x = np.asarray(inputs["x"], np.float32)
    b1, b2 = inputs["b1"], inputs["b2"]
    bp1, bp2 = inputs["bp1"], inputs["bp2"]
    kws, nachs, nbchs, per_core, shared = _host_prep(
        x, inputs["edge_index"], inputs["W1"], inputs["W2"], inputs["Wp1"],
        inputs["Wp2"],
    )

    has_b = {
        "b1": bool(np.any(np.asarray(b1))),
        "b2": bool(np.any(np.asarray(b2))),
        "bp1": bool(np.any(np.asarray(bp1))),
        "bp2": bool(np.any(np.asarray(bp2))),
    }
    nc = _build_program(
        kws, nachs, nbchs, has_b["b1"], has_b["b2"], has_b["bp1"], has_b["bp2"]
    )

    in_maps = []
    for c in range(NC):
        m = dict(per_core[c])
        m.update(shared)
        for nm, b in (("b1", b1), ("b2", b2), ("bp1", bp1), ("bp2", bp2)):
            if has_b[nm]:
                m[nm] = np.ascontiguousarray(
                    np.tile(np.asarray(b, np.float32)[None, :], (128, 1))
                )
        in_maps.append(m)

    res = run_bass_kernel_spmd(
        nc, in_maps, core_ids=list(range(NC)), trace=trace, **run_kwargs
    )

    out = np.empty((N, D), np.float32)
    z = np.empty((N, D), np.float32)
    proj = np.empty((N, D), np.float32)
    for c in range(NC):
        r = res.results[c]
        out[c * NPC : (c + 1) * NPC] = r["agg"][:NPC]
        z[c * NPC : (c + 1) * NPC] = r["z"][:NPC]
        proj[c * NPC : (c + 1) * NPC] = r["proj"][:NPC]
    return (out, z, proj), res


def kernel(x, edge_index, W1, b1, W2, b2, Wp1, bp1, Wp2, bp2):
    outs, _ = run(
        {
            "x": x, "edge_index": edge_index, "W1": W1, "b1": b1,
            "W2": W2, "b2": b2, "Wp1": Wp1, "bp1": bp1,
            "Wp2": Wp2, "bp2": bp2,
        }
    )
    return outs


# revision 38
# speedup vs baseline: 1.0399x; 1.0399x over previous
"""GCN encoder (2x GCNConv + MLP proj head) on 8 Trainium2 NeuronCores.

Strategy: shard nodes across the 8 cores (1250/core, padded to 1280).
The symmetric GCN norm dis[src]*dis[dst] factors into per-node pre/post
scaling, so each aggregation round is: per-core dense matmul (X@W, bf16,
f32 PSUM) + dis-scale -> AllGather of the scaled features -> per 128-dst
window: dma_gather of deduped source rows + host-precomputed one-hot
scatter matmuls accumulating segment sums in PSUM -> self-loop term on
the DVE -> dis post-scale on the scalar engine.

Both feature AllGathers run in HALVES bound to SEPARATE dram tensors
(h?A/h?B): the first h1 half fires right after phase-A window 4 (it
absorbs the runtime's first-CC barrier, which ends only ~15-25us after
the first trigger), and each window's gather pieces are split at the
A/B source boundary (host pads each region per core) so A-pieces gate
only on the A AllGather. Per-window chunk counts are variable
(max-over-cores per half) instead of one global k.

Round 2 runs in fp8e4: h2' is quantized once (out rel err ~1.8e-2 <
the 2e-2 gate), halving the h2 AllGathers and round-2 gather bytes,
with DoubleRow scatter matmuls at 2x PE rate. The S table ships as fp8
(counts exact) and is DVE-cast to bf16 per window for round 1. The
proj head's matmuls interleave into round-2 windows where the PE would
otherwise idle on gathers.
"""
import json

import numpy as np
import ml_dtypes

N = 10000
E = 160000
D = 512
NC = 8
NPC = N // NC  # 1250 nodes per core
CH = 10  # 128-node chunks / windows per core
NPAD = CH * 128  # 1280
HALF = NPAD // 2  # 640

_BF16 = ml_dtypes.bfloat16
_F8 = ml_dtypes.float8_e4m3

_WAIT_SPLIT_DONE = False


def _install_wait_split():
    """This container's walrus rejects instructions with >1 sync wait.
    Hoist extra waits onto single-wait Drain instructions just before the
    instruction on the same engine (same sequencer => same semantics)."""
    global _WAIT_SPLIT_DONE
    if _WAIT_SPLIT_DONE:
        return
    _WAIT_SPLIT_DONE = True
    import concourse.bass as bass

    orig = bass.Bass.to_json_bytes

    def _split_block(instructions):
        out = []
        changed = False
        for inst in instructions:
            sync = inst.get("sync_info")
            waits = (sync or {}).get("on_wait") or []
            if len(waits) > 1:
                changed = True
                for j, w in enumerate(waits[:-1]):
                    out.append(
                        {
                            "engine": inst["engine"],
                            "ins": [],
                            "name": f"{inst['name']}-wsplit{j}",
                            "opcode": "Drain",
                            "outs": [],
                            "sync_info": {"on_update": [], "on_wait": [w]},
                        }
                    )
                sync["on_wait"] = waits[-1:]
            out.append(inst)
        return out, changed

    def to_json_bytes(self):
        js = json.loads(orig(self))
        stack = [js]
        while stack:
            d = stack.pop()
            if isinstance(d, dict):
                if "instructions" in d:
                    new, changed = _split_block(d["instructions"])
                    if changed:
                        d["instructions"] = new
                for v in d.values():
                    if isinstance(v, (dict, list)):
                        stack.append(v)
            elif isinstance(d, list):
                stack.extend(d)
        return json.dumps(js).encode()

    bass.Bass.to_json_bytes = to_json_bytes


def _pieces_for(nach, nbch, gmax):
    """Gather pieces for one window: (chunk_offset, nchunks, half) with A
    chunks [0, nach) then B chunks [nach, nach+nbch), grouped by gmax."""
    out = []
    a = 0
    while a < nach:
        nk = min(gmax, nach - a)
        out.append((a, nk, 0))
        a += nk
    tot = nach + nbch
    while a < tot:
        nk = min(gmax, tot - a)
        out.append((a, nk, 1))
        a += nk
    return out


def _build_program(kws, nachs, nbchs, has_b1, has_b2, has_bp1, has_bp2):
    import concourse.bass as bass
    import concourse.tile as tile
    from concourse import mybir
    from concourse.library_config import mlp
    from concourse.library_overlay import lower_extended_insts
    from concourse.tile_rust import add_dep_helper

    f32 = mybir.dt.float32
    bf16 = mybir.dt.bfloat16
    f8 = mybir.dt.float8e4
    i16 = mybir.dt.int16
    ACTF = mybir.ActivationFunctionType

    TOT = sum(kws)
    KMAX = max(kws)
    offs = np.concatenate([[0], np.cumsum(kws)])[:-1].tolist()
    # round-1: 2-chunk (256-desc, 256KB) pieces; round-2 fp8: 4-chunk
    # (512-desc, 256KB). Bigger pieces hit SWDGE data backpressure.
    pieces1 = [_pieces_for(nachs[w], nbchs[w], 2) for w in range(CH)]
    pieces2 = [_pieces_for(nachs[w], nbchs[w], 4) for w in range(CH)]

    nc = bass.Bass(num_swdge_queues=4)

    # ---- external inputs (per-core layouts prepared on host) ----
    xt_ext = nc.dram_tensor("xt", [128, 4 * NPAD], bf16, kind="ExternalInput")
    w_ext = {
        nm: nc.dram_tensor(nm, [128, 4 * D], bf16, kind="ExternalInput")
        for nm in ("w1t", "w2t", "wp1t", "wp2t")
    }
    dis_ext = nc.dram_tensor("dis", [128, CH], f32, kind="ExternalInput")
    idx_ext = nc.dram_tensor("idx16", [128, TOT * 8], i16, kind="ExternalInput")
    # S (one-hot counts) ships as fp8e4 (counts are small ints, exact):
    # round 2 consumes it directly (fp8 DoubleRow matmuls); round 1 casts
    # per-window to bf16 on the DVE. Halves the dominant input load.
    s_ext = nc.dram_tensor("stab", [128, TOT * 128], f8, kind="ExternalInput")
    ident_ext = nc.dram_tensor("ident", [128, 128], bf16, kind="ExternalInput")
    b_ext = {}
    for nm, has in (
        ("b1", has_b1),
        ("b2", has_b2),
        ("bp1", has_bp1),
        ("bp2", has_bp2),
    ):
        if has:
            b_ext[nm] = nc.dram_tensor(nm, [128, D], f32, kind="ExternalInput")

    # ---- external outputs ----
    # z in bf16: quantization ~2^-9 rel, far under the 2e-2 gate.
    z_out = nc.dram_tensor("z", [NPAD, D], bf16, kind="ExternalOutput")
    out_out = nc.dram_tensor("agg", [NPAD, D], f32, kind="ExternalOutput")
    proj_out = nc.dram_tensor("proj", [NPAD, D], bf16, kind="ExternalOutput")

    # ---- internal DRAM ----
    # Separate tensors per AllGather half: exact Tile deps (A-pieces gate
    # only on the A AllGather; the AG trigger waits only its own shard).
    h1sA = nc.dram_tensor("h1sA", [HALF, D], bf16)
    h1sB = nc.dram_tensor("h1sB", [HALF, D], bf16)
    h1A = nc.dram_tensor("h1A", [NC * HALF, D], bf16, addr_space="Shared")
    h1B = nc.dram_tensor("h1B", [NC * HALF, D], bf16, addr_space="Shared")
    h2sA = nc.dram_tensor("h2sA", [HALF, D], f8)
    h2sB = nc.dram_tensor("h2sB", [HALF, D], f8)
    h2A = nc.dram_tensor("h2A", [NC * HALF, D], f8, addr_space="Shared")
    h2B = nc.dram_tensor("h2B", [NC * HALF, D], f8, addr_space="Shared")
    dum_in = nc.dram_tensor("dum_in", [128, 2], f32)
    dum_out = nc.dram_tensor("dum_out", [128, 16], f32, addr_space="Shared")

    core_ids = list(range(NC))

    with tile.TileContext(nc) as tc:
        with (
            tc.tile_pool(name="const", bufs=1) as cpool,
            tc.tile_pool(name="work", bufs=3) as wpool,
            tc.tile_pool(name="gat", bufs=5) as gpool,
            tc.tile_pool(name="sc", bufs=3) as scpool,
            tc.tile_pool(name="tp", bufs=1) as tpool,
            tc.tile_pool(name="psA", bufs=2, space="PSUM") as psA,
            tc.tile_pool(name="psB", bufs=4, space="PSUM") as psB,
        ):
            # Dep-free dummy first collective, triggered ~11us in: the
            # runtime's first-CC barrier ends only ~25us after the first
            # trigger, so binding it to this tiny AllGather instead of the
            # h1 AG-A (trigger ~35us) starts the real AGs ~25us earlier.
            dum_t = cpool.tile([128, 2], f32, name="dum_sb")
            nc.vector.memset(dum_t[:], 0.0)
            nc.sync.dma_start(dum_in[:], dum_t[:])
            nc.gpsimd.collective_compute(
                "AllGather",
                mybir.AluOpType.bypass,
                ins=[dum_in[:]],
                outs=[dum_out[:]],
                replica_groups=[list(range(NC))],
            )
            lib_inst = nc.gpsimd.load_library(mlp)
            # one shared register per distinct gather size (to_reg per call
            # would exhaust the Pool register file at 60 gathers)
            sizes = sorted(
                {nk for pl in pieces1 + pieces2 for (_, nk, _) in pl}
            )
            nidx_regs = {nk: nc.gpsimd.to_reg(nk * 128) for nk in sizes}

            # ---- phase-A-critical loads, split across the sync + scalar
            # HWDGE rings (each sustains only ~65-115GB/s): w1t + xt halves
            # + dis land first so the first matmul can go ~15us in.
            w_t = {}
            w_t["w1t"] = cpool.tile([128, 4 * D], bf16, tag="w1t", name="w1t")
            nc.sync.dma_start(w_t["w1t"][:], w_ext["w1t"][:])
            xt_t = cpool.tile([128, 4 * NPAD], bf16)
            XH = 2 * NPAD
            nc.scalar.dma_start(xt_t[:, :XH], xt_ext[:, :XH])
            nc.sync.dma_start(xt_t[:, XH:], xt_ext[:, XH:])
            dis_t = cpool.tile([128, CH], f32)
            nc.sync.dma_start(dis_t[:], dis_ext[:])
            # ---- bulk loads on the scalar HWDGE ring (parallel to above;
            # keep the gpsimd queue free so the AG trigger isn't delayed) ----
            idx_t = cpool.tile([128, TOT * 8], i16)
            nc.scalar.dma_start(idx_t[:], idx_ext[:])
            s8_t = cpool.tile([128, TOT * 128], f8)
            nc.scalar.dma_start(s8_t[:], s_ext[:])
            ident_t = cpool.tile([128, 128], bf16)
            nc.scalar.dma_start(ident_t[:], ident_ext[:])
            for nm in ("w2t", "wp1t", "wp2t"):
                w_t[nm] = cpool.tile([128, 4 * D], bf16, tag=nm, name=nm)
                nc.scalar.dma_start(w_t[nm][:], w_ext[nm][:])
            b_t = {}
            for nm in b_ext:
                b_t[nm] = cpool.tile([128, D], f32, tag=nm, name=nm + "_bc")
                nc.scalar.dma_start(b_t[nm][:], b_ext[nm][:])

            # persistent scaled-feature chunks (self-loop term source)
            h1p_t = cpool.tile([128, CH * D], bf16)
            h2p_t = cpool.tile([128, CH * D], bf16)

            def dense_layer(lhs_tiles, w_name, m, dep=None):
                ps = psA.tile([128, D], f32, tag="dense")
                for kk in range(4):
                    mi = nc.tensor.matmul(
                        ps[:],
                        lhs_tiles(kk, m),
                        w_t[w_name][:, kk * D : (kk + 1) * D],
                        start=(kk == 0),
                        stop=(kk == 3),
                    )
                    if kk == 0 and dep is not None:
                        add_dep_helper(
                            getattr(mi, "ins", mi),
                            getattr(dep, "ins", dep),
                            reason="pin proj head to its window",
                        )
                return ps

            def xt_tile(kk, m):
                return xt_t[:, kk * NPAD + m * 128 : kk * NPAD + (m + 1) * 128]

            def scale_to(dst_ap, ps, m, bias_name):
                """dst = dis_m * (ps + bias) via ACT (bias pre-add on DVE).
                Returns the pre-scale source for further ACT copies."""
                if bias_name in b_t:
                    tmp = wpool.tile([128, D], f32, tag="btmp")
                    nc.vector.tensor_tensor(
                        tmp[:], ps[:], b_t[bias_name][:], op=mybir.AluOpType.add
                    )
                    src = tmp
                else:
                    src = ps
                nc.scalar.activation(
                    dst_ap, src[:], ACTF.Copy, scale=dis_t[:, m : m + 1]
                )
                return src

            # ---- phase A: H1' = dis * (X @ W1 + b1), own nodes ----
            # Windows 0-4 land in h1sA, 5-9 in h1sB; the A AllGather fires
            # right after window 4 (absorbs the first-CC barrier).
            for m in range(CH):
                ps = dense_layer(xt_tile, "w1t", m)
                scale_to(h1p_t[:, m * D : (m + 1) * D], ps, m, "b1")
                sh, r0 = (h1sA, m * 128) if m < 5 else (h1sB, (m - 5) * 128)
                nc.sync.dma_start(
                    sh[r0 : r0 + 128, :], h1p_t[:, m * D : (m + 1) * D]
                )
                if m == 4:
                    nc.gpsimd.collective_compute(
                        "AllGather",
                        mybir.AluOpType.bypass,
                        ins=[h1sA[0:HALF, :]],
                        outs=[h1A[0 : NC * HALF, :]],
                        replica_groups=[core_ids],
                    )
            nc.gpsimd.collective_compute(
                "AllGather",
                mybir.AluOpType.bypass,
                ins=[h1sB[0:HALF, :]],
                outs=[h1B[0 : NC * HALF, :]],
                replica_groups=[core_ids],
            )

            def one_gather(src_ap, off, a, nk, g_ap, qn):
                """Gather chunks [a, a+nk) of a window's table (flat chunk
                offset `off`+a in the idx/S tables) into g_ap."""
                gi = nc.gpsimd.dma_gather(
                    g_ap,
                    src_ap,
                    idx_t[:, (off + a) * 8 : (off + a + nk) * 8],
                    num_idxs=nk * 128,
                    num_idxs_reg=nidx_regs[nk],
                    elem_size=D,
                    single_packet=True,
                    queue_num=qn,
                )
                add_dep_helper(
                    getattr(gi, "ins", gi),
                    getattr(lib_inst, "ins", lib_inst),
                    reason="mlp library before dma_gather",
                )

            def issue_gathers(srcA, srcB, w, g_t, pieces):
                for hh, (a, nk, hb) in enumerate(pieces[w]):
                    src = srcA[:] if hb == 0 else srcB[:]
                    one_gather(
                        src, offs[w], a, nk, g_t[:, a : a + nk, :], hh % 4
                    )

            def self_add(ps, selfsrc_t, w):
                """Self-loop term on DVE (frees the PE of identity matmuls)."""
                tmp = wpool.tile([128, D], f32, tag="selfadd", bufs=3)
                nc.vector.tensor_tensor(
                    tmp[:],
                    ps[:],
                    selfsrc_t[:, w * D : (w + 1) * D],
                    op=mybir.AluOpType.add,
                )
                return tmp

            def agg_round1(srcA, srcB, selfsrc_t):
                """bf16 scatter: per window, DVE-cast the fp8 S chunk to bf16
                then accumulate k_w chunk matmuls."""
                for w in range(CH):
                    k_w = kws[w]
                    g_t = gpool.tile([128, KMAX, D], bf16, tag="g", name=f"g{w}")
                    issue_gathers(srcA, srcB, w, g_t, pieces1)
                    s_c = scpool.tile(
                        [128, KMAX * 128], bf16, tag="sc", name=f"sc{w}"
                    )
                    nc.vector.tensor_copy(
                        s_c[:, : k_w * 128],
                        s8_t[:, offs[w] * 128 : (offs[w] + k_w) * 128],
                    )
                    ps = psB.tile([128, D], f32, tag="agg", name=f"agg{w}")
                    for k in range(k_w):
                        nc.tensor.matmul(
                            ps[:],
                            s_c[:, k * 128 : (k + 1) * 128],
                            g_t[:, k, :],
                            start=(k == 0),
                            stop=(k == k_w - 1),
                        )
                    yield w, self_add(ps, selfsrc_t, w)

            s8_pairs = s8_t[:].rearrange("p (c k) -> p c k", k=128)

            def agg_round2(srcA, srcB, selfsrc_t, pre=None):
                """fp8 DoubleRow scatter: pairs of 128-src chunks per matmul
                at 2x PE rate; gathers move half the bytes."""
                g_full = None
                for w in range(CH):
                    k_w = kws[w]
                    # fp8 tiles are half the round-1 bf16 size: pack TWO
                    # windows per shared-tag buffer, so all 10 round-2
                    # windows fit in 5 bufs with no rotation — every gather
                    # can pre-issue as soon as the h2 AG halves land.
                    if w % 2 == 0:
                        g_full = gpool.tile(
                            [128, 2 * KMAX, D], f8, tag="g", name=f"g8_{w}"
                        )
                    hw_ = (w % 2) * KMAX
                    g_t = g_full[:, hw_ : hw_ + KMAX, :]
                    issue_gathers(srcA, srcB, w, g_t, pieces2)
                    if pre is not None:
                        pre(w)
                    ps = psB.tile([128, D], f32, tag="agg", name=f"agg{w}")
                    np_ = k_w // 2
                    for c in range(np_):
                        nc.tensor.matmul(
                            ps[:],
                            s8_pairs[
                                :, offs[w] + 2 * c : offs[w] + 2 * c + 2, :
                            ],
                            g_t[:, 2 * c : 2 * c + 2, :],
                            start=(c == 0),
                            stop=(c == np_ - 1),
                            perf_mode=mybir.MatmulPerfMode.DoubleRow,
                        )
                    yield w, self_add(ps, selfsrc_t, w)

            # ---- fused round 1 + L2, pipelined per window ----
            # zt window-major [128, w, kk, 128]
            zt_t = tpool.tile([128, CH * 4 * 128], bf16, tag="zt")
            zt_v = zt_t[:].rearrange("p (w k n) -> p w k n", k=4, n=128)

            def proj_head(w):
                ps3 = dense_layer(lambda kk, m: zt_v[:, m, kk, :], "wp1t", w)
                p1_b = wpool.tile([128, D], bf16, tag="p1")
                if "bp1" in b_t:
                    btmp = wpool.tile([128, D], f32, tag="btmp")
                    nc.vector.tensor_tensor(
                        btmp[:], ps3[:], b_t["bp1"][:], op=mybir.AluOpType.add
                    )
                    nc.vector.tensor_scalar(
                        p1_b[:], btmp[:], 0.0, None, op0=mybir.AluOpType.max
                    )
                else:
                    nc.vector.tensor_scalar(
                        p1_b[:], ps3[:], 0.0, None, op0=mybir.AluOpType.max
                    )
                psT2 = psA.tile([128, 4, 128], bf16, tag="tr", name=f"tr2{w}")
                for kk in range(4):
                    nc.tensor.transpose(
                        psT2[:, kk, :],
                        p1_b[:, kk * 128 : (kk + 1) * 128],
                        ident_t[:],
                    )
                # p1^T is consumed immediately by proj2 — rotating tile
                p1c = scpool.tile([128, 4, 128], bf16, tag="p1c", name=f"p1c{w}")
                nc.vector.tensor_copy(p1c[:], psT2[:])
                ps4 = dense_layer(lambda kk, m: p1c[:, kk, :], "wp2t", w)
                pj_t = wpool.tile([128, D], bf16, tag="pj")
                if "bp2" in b_t:
                    nc.vector.tensor_tensor(
                        pj_t[:], ps4[:], b_t["bp2"][:], op=mybir.AluOpType.add
                    )
                else:
                    nc.vector.tensor_copy(pj_t[:], ps4[:])
                nc.sync.dma_start(proj_out[w * 128 : (w + 1) * 128, :], pj_t[:])

            for w, tmp in agg_round1(h1A, h1B, h1p_t):
                z_b = wpool.tile([128, D], bf16, tag="zb")
                nc.scalar.activation(
                    z_b[:], tmp[:], ACTF.Copy, scale=dis_t[:, w : w + 1]
                )
                nc.sync.dma_start(z_out[w * 128 : (w + 1) * 128, :], z_b[:])
                # transpose z chunk into zt columns; relu'd copy into rt
                psT = psA.tile([128, 4, 128], bf16, tag="tr", name=f"tr{w}")
                for kk in range(4):
                    nc.tensor.transpose(
                        psT[:, kk, :],
                        z_b[:, kk * 128 : (kk + 1) * 128],
                        ident_t[:],
                    )
                zt_cols = zt_v[:, w, :, :]
                # rt (relu'd z^T) is consumed by L2 in this same window, so a
                # rotating tile suffices (saves 2.5MB of SBUF vs NPAD-wide)
                rt_w = wpool.tile([128, 4, 128], bf16, tag="rtw")
                nc.vector.tensor_copy(zt_cols, psT[:])
                nc.vector.tensor_scalar(
                    rt_w[:], psT[:], 0.0, None, op0=mybir.AluOpType.max
                )
                # L2 for this node chunk -> H2' shard (bf16 self copy + fp8
                # collective copy)
                ps2 = dense_layer(lambda kk, m: rt_w[:, kk, :], "w2t", w)
                src2 = scale_to(h2p_t[:, w * D : (w + 1) * D], ps2, w, "b2")
                h2f8 = wpool.tile([128, D], f8, tag="h2f8")
                nc.scalar.activation(
                    h2f8[:], src2[:], ACTF.Copy, scale=dis_t[:, w : w + 1]
                )
                sh2, r0 = (h2sA, w * 128) if w < 5 else (h2sB, (w - 5) * 128)
                nc.sync.dma_start(sh2[r0 : r0 + 128, :], h2f8[:])
                if w == 4:
                    nc.gpsimd.collective_compute(
                        "AllGather",
                        mybir.AluOpType.bypass,
                        ins=[h2sA[0:HALF, :]],
                        outs=[h2A[0 : NC * HALF, :]],
                        replica_groups=[core_ids],
                    )
            nc.gpsimd.collective_compute(
                "AllGather",
                mybir.AluOpType.bypass,
                ins=[h2sB[0:HALF, :]],
                outs=[h2B[0 : NC * HALF, :]],
                replica_groups=[core_ids],
            )

            # ---- phase E: round 2 -> out, proj head interleaved per window
            # (proj depends only on zt, so its matmuls fill the tensor engine
            # while the h2 AllGather half and the window's gathers land) ----
            for w, tmp in agg_round2(h2A, h2B, h2p_t, pre=proj_head):
                o_f = wpool.tile([128, D], f32, tag="of")
                nc.scalar.activation(
                    o_f[:], tmp[:], ACTF.Copy, scale=dis_t[:, w : w + 1]
                )
                nc.sync.dma_start(out_out[w * 128 : (w + 1) * 128, :], o_f[:])

    lower_extended_insts(nc)
    return nc


def _host_prep(x, edge_index, W1, W2, Wp1, Wp2):
    src = np.asarray(edge_index[0], np.int64)
    dst = np.asarray(edge_index[1], np.int64)

    # degree includes self loops (norm definition), but self edges are
    # handled on-device via the self-add, not the gather.
    deg = (np.bincount(np.concatenate([dst, np.arange(N)]), minlength=N)).astype(
        np.float32
    )
    dis = (1.0 / np.sqrt(np.maximum(deg, 1.0))).astype(np.float32)

    owner = src // NPC
    local = src - owner * NPC
    # AllGather halves land rank-major per half: row = owner*HALF + local
    # within each half tensor (A: local<HALF; B: local-HALF).
    in_b = local >= HALF
    gather_row = np.where(
        in_b, NC * HALF + owner * HALF + (local - HALF), owner * HALF + local
    )

    dst_core = dst // NPC
    dst_local = dst - dst_core * NPC  # [0, 1250)
    win = dst_local // 128
    dloc = dst_local - win * 128

    order = np.lexsort((dst_local, dst_core))
    g_sorted = gather_row[order]
    dc = dst_core[order]
    wn = win[order]
    dl = dloc[order]

    counts = np.zeros((NC, CH), np.int64)
    np.add.at(counts, (dc, wn), 1)
    flat_counts = counts.reshape(-1)
    starts = np.concatenate([[0], np.cumsum(flat_counts)])[:-1].reshape(NC, CH)

    # dedup per (core, window); split at the A/B half boundary (sources
    # sorted ascending so A rows come first)
    uniq = {}
    cntA = np.zeros((NC, CH), np.int64)
    cntB = np.zeros((NC, CH), np.int64)
    for c in range(NC):
        for w in range(CH):
            s0, n = starts[c, w], counts[c, w]
            rows = g_sorted[s0 : s0 + n]
            dd = dl[s0 : s0 + n]
            u, inv = np.unique(rows, return_inverse=True)
            a = int(np.searchsorted(u, NC * HALF))
            uniq[(c, w)] = (u, inv, dd, a)
            cntA[c, w] = a
            cntB[c, w] = len(u) - a

    # per-window shared chunk counts: each half padded (duplicate row 0,
    # zero S columns) to the max over cores; total kept even for the
    # round-2 DoubleRow chunk pairs.
    nachs, nbchs, kws = [], [], []
    for w in range(CH):
        na = int(np.ceil(cntA[:, w].max() / 128))
        nb = int(np.ceil(cntB[:, w].max() / 128))
        if (na + nb) % 2:
            nb += 1
        nachs.append(na)
        nbchs.append(nb)
        kws.append(na + nb)
    offs = np.concatenate([[0], np.cumsum(kws)])[:-1]
    TOT = int(sum(kws))

    per_core = []
    for c in range(NC):
        idx_flat = np.zeros((TOT * 128,), np.int64)
        s_tab = np.zeros((TOT * 128, 128), np.float32)
        for w in range(CH):
            u, inv, dd, a = uniq[(c, w)]
            base = offs[w] * 128
            nA = nachs[w] * 128
            # A sources at [base, base+lenA), B at [base+nA, ...): indices
            # rebased per half tensor; inv remapped to the padded layout.
            idx_flat[base : base + a] = u[:a]
            idx_flat[base + nA : base + nA + (len(u) - a)] = (
                u[a:] - NC * HALF
            )
            pos = np.where(inv < a, inv, nA + (inv - a))
            np.add.at(s_tab, (base + pos, dd), 1.0)

        wlen16 = TOT * 128 // 16
        iw = idx_flat.reshape(wlen16, 16).T  # [16, TOT*8]
        idx16 = np.ascontiguousarray(
            np.tile(iw, (8, 1)).astype(np.int16)
        )

        # stab: [128, TOT*128]; col (off+k)*128+d, part p = S[(off+k)*128+p, d]
        stab = (
            s_tab.reshape(TOT, 128, 128).transpose(1, 0, 2).reshape(128, -1)
        )
        stab = np.ascontiguousarray(stab).astype(_F8)

        xc = np.zeros((NPAD, D), np.float32)
        xc[:NPC] = x[c * NPC : (c + 1) * NPC]
        xt = xc.T.reshape(4, 128, NPAD).transpose(1, 0, 2).reshape(128, -1)
        xt = np.ascontiguousarray(xt).astype(_BF16)

        dis_c = np.zeros((NPAD,), np.float32)
        dis_c[:NPC] = dis[c * NPC : (c + 1) * NPC]
        dis_t = np.ascontiguousarray(dis_c.reshape(CH, 128).T, np.float32)

        per_core.append(
            {"xt": xt, "idx16": idx16, "stab": stab, "dis": dis_t}
        )

    def wtile(W):
        wt = (
            np.asarray(W, np.float32)
            .reshape(4, 128, D)
            .transpose(1, 0, 2)
            .reshape(128, -1)
        )
        return np.ascontiguousarray(wt).astype(_BF16)

    shared = {
        "w1t": wtile(W1),
        "w2t": wtile(W2),
        "wp1t": wtile(Wp1),
        "wp2t": wtile(Wp2),
        "ident": np.eye(128, dtype=np.float32).astype(_BF16),
    }
    return kws, nachs, nbchs, per_core, shared


def run(inputs, trace=False, **run_kwargs):
    """Build + run; returns ((out, z, proj), BassKernelResults)."""
    _install_wait_split()
    from concourse.bass_utils import run_bass_kernel_spmd

    x = np.asarray(inputs["x"], np.float32)
    b1, b2 = inputs["b1"], inputs["b2"]
    bp1, bp2 = inputs["bp1"], inputs["bp2"]
    kws, nachs, nbchs, per_core, shared = _host_prep(
        x, inputs["edge_index"], inputs["W1"], inputs["W2"], inputs["Wp1"],
        inputs["Wp2"],
    )

    has_b = {
        "b1": bool(np.any(np.asarray(b1))),
        "b2": bool(np.any(np.asarray(b2))),
        "bp1": bool(np.any(np.asarray(bp1))),
        "bp2": bool(np.any(np.asarray(bp2))),
    }
    nc = _build_program(
        kws, nachs, nbchs, has_b["b1"], has_b["b2"], has_b["bp1"], has_b["bp2"]
    )

    in_maps = []
    for c in range(NC):
        m = dict(per_core[c])
        m.update(shared)
        for nm, b in (("b1", b1), ("b2", b2), ("bp1", bp1), ("bp2", bp2)):
            if has_b[nm]:
                m[nm] = np.ascontiguousarray(
                    np.tile(np.asarray(b, np.float32)[None, :], (128, 1))
                )
        in_maps.append(m)

    res = run_bass_kernel_spmd(
        nc, in_maps, core_ids=list(range(NC)), trace=trace, **run_kwargs
    )

    out = np.empty((N, D), np.float32)
    z = np.empty((N, D), np.float32)
    proj = np.empty((N, D), np.float32)
    for c in range(NC):
        r = res.results[c]
        out[c * NPC : (c + 1) * NPC] = r["agg"][:NPC]
        z[c * NPC : (c + 1) * NPC] = r["z"][:NPC]
        proj[c * NPC : (c + 1) * NPC] = r["proj"][:NPC]
    return (out, z, proj), res


def kernel(x, edge_index, W1, b1, W2, b2, Wp1, bp1, Wp2, bp2):
    outs, _ = run(
        {
            "x": x, "edge_index": edge_index, "W1": W1, "b1": b1,
            "W2": W2, "b2": b2, "Wp1": Wp1, "bp1": bp1,
            "Wp2": Wp2, "bp2": bp2,
        }
    )
    return outs
